# revision 29
# baseline (speedup 1.0000x reference)
"""Trainium2 Bass kernel for batched bilinear (general) attention.

Reference computation (all fp32):
    psi = einsum("bth,ah->bta", h_enc, W_psi) + b_psi        # [B, T, A]
    phi = einsum("qbh,ah->qba", h_dec, W_phi) + b_phi        # [Q, B, A]
    e   = einsum("bta,qba->btq", psi, phi)                   # [B, T, Q]
    a   = softmax(e, axis=1)                                 # over T
    c   = einsum("bth,btq->bqh", h_enc, a)                   # [B, Q, H]

Algebraic refactor: e[b,t,q] = enc_t . M . dec_q + enc_t . u + (per-q const)
with M = W_psi^T @ W_phi [H,H], u = W_psi^T @ b_phi.  Per-q-column constants
are invariant under the softmax over t, so they are dropped.  The host folds
the weights into Z[b] = M @ dec_b^T + u [H, Q] (tiny); the device computes
e^T = Z^T @ enc^T, the softmax over T, and c = softmax(e)^T @ enc.

All device operands are fp16 (rel-err budget 2e-2; measured ~1e-2): the
fp32-correction passes of earlier revisions are dropped, which cuts HBM
traffic to the 16 MB/core floor (enc read twice: once transposed for the
e-phase H-contraction, once natural for the c-phase T-contraction) and
halves e-phase PE work.

Sharding: data-parallel over batch B=16 across 8 cores (2 batches/core),
no collectives.
"""

import functools
import os
import sys

import numpy as np

for _p in ("/opt/trn_rl_repo", "/root/.axon_site/_ro/trn_rl_repo"):
    if os.path.isdir(_p) and _p not in sys.path:
        sys.path.append(_p)

B, T, Q, H = 16, 2048, 64, 1024
NCORES = 8
BL = B // NCORES  # batches per core
KT = H // 128  # 8 contraction tiles for e
NT = T // 128  # 16 t-tiles
NCH = T // 512  # 4 chunks of 512 along T (e-phase PSUM banks)

GE = int(os.environ.get("ATTN_GE", "2"))  # k-tiles per encT DMA (1 MB @ 2)
GC = int(os.environ.get("ATTN_GC", "2"))  # t-tiles per encN DMA (512 KB @ 2)
# phase schedule: per-batch ("batch") or E-phases first ("phase")
ORDER = os.environ.get("ATTN_ORDER", "phase")


@functools.lru_cache(maxsize=8)
def _build(loop_n: int = 1, ge: int = GE, gc: int = GC, order: str = ORDER):
    import contextlib

    import concourse.mybir as mybir
    import concourse.tile as tile
    from concourse import bacc
    from concourse.bass import ts
    from concourse.masks import make_identity

    f32 = mybir.dt.float32
    f16 = mybir.dt.float16

    nc = bacc.Bacc(
        "TRN2",
        target_bir_lowering=False,
        debug=False,
        enable_asserts=False,
        num_devices=NCORES,
    )

    # host-interleaved tiled layouts: each partition's slice of one DMA is a
    # single contiguous run, keeping every descriptor >= 512 B
    encT_d = nc.dram_tensor("encT", [BL, KT // ge, 128, ge * T], f16, kind="ExternalInput")
    encN_d = nc.dram_tensor("encN", [BL, NT // gc, 128, gc * H], f16, kind="ExternalInput")
    z_d = nc.dram_tensor("z", [BL, 128, KT * Q], f16, kind="ExternalInput")
    c_d = nc.dram_tensor("c", [BL, Q, H], f32, kind="ExternalOutput")

    with tile.TileContext(nc) as tc:
        with (
            tc.tile_pool(name="encT", bufs=BL * (KT // ge)) as p_encT,
            tc.tile_pool(name="encN", bufs=BL * (NT // gc)) as p_encN,
            tc.tile_pool(name="z", bufs=2) as p_z,
            tc.tile_pool(name="pT", bufs=2) as p_pT,
            tc.tile_pool(name="eT", bufs=2) as p_eT,
            tc.tile_pool(name="pN", bufs=2) as p_pN,
            tc.tile_pool(name="outs", bufs=2) as p_out,
            tc.tile_pool(name="stats", bufs=12) as p_stats,
            tc.tile_pool(name="singles", bufs=1) as p_singles,
            tc.tile_pool(name="ps", bufs=8, space="PSUM") as ps,
        ):
            ident = p_singles.tile([64, 64], f16)
            make_identity(nc, ident)

            # all big enc DMAs go on sync (HWDGE pipelines its queue, so one
            # engine sustains back-to-back transfers) in consumption order —
            # a single FIFO makes the device drain globally in-order
            def enc_dma(out, in_):
                nc.sync.dma_start(out=out, in_=in_)

            loop_ctx = (
                tc.For_i(0, loop_n, 1) if loop_n > 1 else contextlib.nullcontext()
            )
            with loop_ctx:
                z_ts, e_ps, sm, pNs = [], {}, {}, {}
                for b in range(BL):
                    z_t = p_z.tile([128, KT, Q], f16, tag="z")
                    nc.scalar.dma_start(out=z_t[:], in_=z_d.ap()[b])
                    z_ts.append(z_t)

                def e_phase(b, kks=None):
                    if b not in e_ps:
                        e_ps[b] = [
                            ps.tile([64, 512], f32, tag="ps", name=f"e_ps_{b}_{ci}")
                            for ci in range(NCH)
                        ]
                    e_pss = e_ps[b]
                    for kk in kks if kks is not None else range(KT // ge):
                        encT_g = p_encT.tile([128, ge * T], f16, tag="encT")
                        enc_dma(encT_g[:], encT_d.ap()[b, kk])
                        # k-contiguous per chunk bank (HAM stays warm)
                        for ci in range(NCH):
                            for g in range(ge):
                                k = kk * ge + g
                                nc.tensor.matmul(
                                    e_pss[ci][:],
                                    lhsT=z_ts[b][:, k, :],
                                    rhs=encT_g[:, ts(g * NCH + ci, 512)],
                                    start=(k == 0),
                                    stop=(k == KT - 1),
                                )

                def softmax(b):
                    # softmax over T directly from the PSUM chunks
                    e_pss = e_ps[b]
                    m4 = p_stats.tile([64, NCH], f32, tag="m4")
                    for ci in range(NCH):
                        nc.vector.reduce_max(
                            out=m4[:, ci : ci + 1], in_=e_pss[ci][:],
                            axis=mybir.AxisListType.X,
                        )
                    negm = p_stats.tile([64, 1], f32, tag="negm")
                    nc.vector.reduce_max(
                        out=negm[:], in_=m4[:], axis=mybir.AxisListType.X, negate=True
                    )
                    pT = p_pT.tile([64, T], f16, tag="pT")
                    ss = p_stats.tile([64, NCH], f32, tag="ss")
                    for ci in range(NCH):
                        nc.scalar.activation(
                            out=pT[:, ts(ci, 512)],
                            in_=e_pss[ci][:],
                            func=mybir.ActivationFunctionType.Exp,
                            bias=negm[:],
                            scale=1.0,
                            accum_out=ss[:, ci : ci + 1],
                        )
                    s1 = p_stats.tile([64, 1], f32, tag="s1")
                    nc.vector.reduce_sum(out=s1[:], in_=ss[:], axis=mybir.AxisListType.X)
                    r = p_stats.tile([64, 1], f32, tag="r")
                    nc.vector.reciprocal(out=r[:], in_=s1[:])
                    sm[b] = (pT, r)

                def t_phase(b):
                    # transpose p^T [64, T] -> p natural tiles [128, 64] fp16
                    pT, _ = sm[b]
                    pN = p_pN.tile([128, NT, Q], f16, tag="pN")
                    pNs[b] = pN
                    for tt in range(NT):
                        tr_ps = ps.tile([128, 64], f16, tag="ps", name=f"tr_{b}_{tt}")
                        nc.tensor.transpose(
                            out=tr_ps[:], in_=pT[:, ts(tt, 128)], identity=ident[:]
                        )
                        # scalar (not DVE) evacuates: DVE's queue carries the
                        # softmax stat chain and would delay these copies
                        nc.scalar.activation(
                            out=pN[:, tt, :],
                            in_=tr_ps[:],
                            func=mybir.ActivationFunctionType.Copy,
                            bias=0.0,
                            scale=1.0,
                        )

                def c_phase(b):
                    # c[b] = p^T @ encN[b] (contract T), scaled by r on evac
                    _, r = sm[b]
                    pN = pNs[b]
                    c_ps0 = ps.tile([64, 512], f32, tag="ps", name=f"c_ps0_{b}")
                    c_ps1 = ps.tile([64, 512], f32, tag="ps", name=f"c_ps1_{b}")
                    for tg in range(NT // gc):
                        encN_t = p_encN.tile([128, gc * H], f16, tag="encN")
                        enc_dma(encN_t[:], encN_d.ap()[b, tg])
                        for g in range(gc):
                            tt = tg * gc + g
                            nc.tensor.matmul(
                                c_ps0[:],
                                lhsT=pN[:, tt, :],
                                rhs=encN_t[:, ts(2 * g, 512)],
                                start=(tt == 0),
                                stop=(tt == NT - 1),
                            )
                            nc.tensor.matmul(
                                c_ps1[:],
                                lhsT=pN[:, tt, :],
                                rhs=encN_t[:, ts(2 * g + 1, 512)],
                                start=(tt == 0),
                                stop=(tt == NT - 1),
                            )

                    # scale-by-r: halves in parallel on DVE + scalar, each
                    # half DMA'd out as soon as it is ready
                    out_t = p_out.tile([64, H], f32, tag="out")
                    nc.vector.tensor_scalar_mul(out_t[:, 0:512], c_ps0[:], r[:])
                    nc.scalar.activation(
                        out=out_t[:, ts(1, 512)],
                        in_=c_ps1[:],
                        func=mybir.ActivationFunctionType.Copy,
                        bias=0.0,
                        scale=r[:],
                    )
                    out_eng = nc.sync if b == BL - 1 else nc.scalar
                    out_eng.dma_start(out=c_d.ap()[b][:, 0:512], in_=out_t[:, 0:512])
                    out_eng.dma_start(out=c_d.ap()[b][:, 512:1024], in_=out_t[:, 512:1024])

                if order == "batch":
                    for b in range(BL):
                        e_phase(b)
                        softmax(b)
                        t_phase(b)
                        c_phase(b)
                elif order == "phase2":
                    # T0's transposes fill E1's inter-group DMA gap
                    e_phase(0)
                    softmax(0)
                    e_phase(1, [0])
                    t_phase(0)
                    e_phase(1, list(range(1, KT // ge)))
                    softmax(1)
                    c_phase(0)
                    t_phase(1)
                    c_phase(1)
                elif order == "phase3":  # both T phases before C0
                    e_phase(0)
                    softmax(0)
                    e_phase(1)
                    softmax(1)
                    t_phase(0)
                    t_phase(1)
                    c_phase(0)
                    c_phase(1)
                else:  # "phase": one long E stream, then T/C
                    e_phase(0)
                    softmax(0)
                    e_phase(1)
                    softmax(1)
                    t_phase(0)
                    c_phase(0)
                    t_phase(1)
                    c_phase(1)

    nc.compile()
    return nc


def _tile_i(x, g):
    """[B, R, W] -> [B, R//(g*128), 128, g*W]: 128-row tiles, g-major free dim."""
    Bn, R, W = x.shape
    G = R // (g * 128)
    return np.ascontiguousarray(
        x.reshape(Bn, G, g, 128, W).transpose(0, 1, 3, 2, 4).reshape(Bn, G, 128, g * W)
    )


def _host_prep(h_enc, h_dec, W_psi, b_psi, W_phi, b_phi):
    h_enc = np.asarray(h_enc, dtype=np.float32)
    W_psi = np.asarray(W_psi, dtype=np.float64)
    W_phi = np.asarray(W_phi, dtype=np.float64)
    b_phi = np.asarray(b_phi, dtype=np.float64)

    # M = W_psi^T @ W_phi [H, H]; u = W_psi^T @ b_phi [H]
    # Z[b, h, q] = sum_k M[h, k] * h_dec[q, b, k] + u[h]
    M = W_psi.T @ W_phi
    u = W_psi.T @ b_phi
    dec_r = np.asarray(h_dec, np.float64).transpose(2, 1, 0).reshape(H, B * Q)
    Z = (M @ dec_r).reshape(H, B, Q).transpose(1, 0, 2) + u[None, :, None]  # [B,H,Q]
    z16 = Z.astype(np.float32).astype(np.float16)
    z_tiled = np.ascontiguousarray(
        z16.reshape(B, KT, 128, Q).transpose(0, 2, 1, 3).reshape(B, 128, KT * Q)
    )

    enc16 = h_enc.astype(np.float16)  # [B, T, H]
    encT16 = np.ascontiguousarray(enc16.transpose(0, 2, 1))  # [B, H, T]
    return {
        "encT": _tile_i(encT16, GE),
        "encN": _tile_i(enc16, GC),
        "z": z_tiled,
    }


def _in_maps(arrays):
    maps = []
    for i in range(NCORES):
        s = slice(i * BL, (i + 1) * BL)
        maps.append({k: v[s] for k, v in arrays.items()})
    return maps


def kernel(h_enc, h_dec, W_psi, b_psi, W_phi, b_phi):
    from concourse.bass_utils import run_bass_kernel_spmd

    arrays = _host_prep(h_enc, h_dec, W_psi, b_psi, W_phi, b_phi)
    nc = _build()
    res = run_bass_kernel_spmd(nc, _in_maps(arrays), core_ids=list(range(NCORES)))
    out = np.concatenate([res.results[i]["c"] for i in range(NCORES)], axis=0)
    return np.ascontiguousarray(out, dtype=np.float32)
